# revision 12
# baseline (speedup 1.0000x reference)
"""Causal self-attention Trainium2 kernel (8 NeuronCores, bf16 compute).

Sharding: core c -> batch b = c//4, head group hg = c%4 (4 heads each).
Each core computes its heads' QKV projections, causal attention, and a
partial output projection yt[d, t] (transposed, bf16). Host sums the 4
partials per batch, transposes, and adds b_proj.

All GEMMs run bf16 at the PE's full rate (fp8 DoubleRow measured only
2x-per-contraction but lost it again to per-instruction PE mode
switches; residual-corrected fp8 costs 1.5x bf16 cycles).  Accuracy
~4e-3 vs the 2e-2 gate.

Device dataflow per core:
  xT [128, KT, S] bf16 resident (host pre-transposed); per head:
    QT/KTt/VT = W.T @ xT (transposed projections, hd on partitions)
    vh = PE-transpose(VT) -> natural [tok, hd] blocks
    per q-span (512): per k-block kj (exact causal cols):
       ST[k,q] = KT_blk.T @ QT   (+ trimask on diagonal blocks, DVE)
       PT = exp(scale*ST)        (ACT, bf16, unnormalized)
       sum[128,q] += ones_sq.T @ PT ; OT~[hd,q] += vh_blk.T @ PT
    OT[h] = OT~ * recip(sum)  (DVE)
  proj: yt[dc, t] = sum_h Wp_h.T @ OT_h -> bf16 -> DRAM

Schedule: head h+1's QKV groups (PE-bound) interleave between the
attention spans of head h (ACT/exp-bound); output-proj chunks
interleave into the last head's spans; head 0's QKV is emitted
kt-major across 6 concurrent PSUM groups so the PE chases the x DMA
stripe-by-stripe instead of waiting for the full 8MB load.
Engine split: ACT exp + half proj copies + wq DMA issue; DVE qkv
bias-copies, trimask, recip, normalize, vh copies + half proj copies.
"""
import numpy as np

B, S, D, H = 2, 2048, 2048, 16
HD = 128
NCORES = 8
HPC = H // (NCORES // B)     # heads per core = 4
NEG = -1e9


def build_nc(S=S, D=D, nh=HPC, span=512):
    import concourse.bass as bass
    import concourse.mybir as mybir
    from concourse import bacc
    from concourse.tile import TileContext

    f32 = mybir.dt.float32
    bf16 = mybir.dt.bfloat16
    KT = D // 128          # contraction tiles for qkv
    TT = S // 128          # token tiles
    NS = S // span         # q spans
    KPS = span // 128      # k-blocks per span
    scale = float(HD) ** -0.5

    nc = bacc.Bacc("TRN2", target_bir_lowering=False, debug=False)
    x_d = nc.dram_tensor("xt", [D, S], bf16, kind="ExternalInput").ap()
    wq_d = nc.dram_tensor("wqkv", [3 * nh * 128, D], bf16,
                          kind="ExternalInput").ap()
    bq_d = nc.dram_tensor("bqkv", [128, 3 * nh], f32, kind="ExternalInput").ap()
    wp_d = nc.dram_tensor("wproj", [nh * 128, D], bf16,
                          kind="ExternalInput").ap()
    tm_d = nc.dram_tensor("trimaskT", [128, 128], f32, kind="ExternalInput").ap()
    id_d = nc.dram_tensor("identb", [128, 128], bf16, kind="ExternalInput").ap()
    oc_d = nc.dram_tensor("ones_sq", [128, 128], bf16, kind="ExternalInput").ap()
    yt_d = nc.dram_tensor("yt", [D, S], bf16, kind="ExternalOutput").ap()

    Act = mybir.ActivationFunctionType
    Alu = mybir.AluOpType

    with TileContext(nc) as tc:
        from contextlib import ExitStack
        with ExitStack() as ctx:
            res = ctx.enter_context(tc.tile_pool(name="res", bufs=1))
            w_p = ctx.enter_context(tc.tile_pool(name="w", bufs=2))
            qk_p = ctx.enter_context(tc.tile_pool(name="qk", bufs=2))
            v_p = ctx.enter_context(tc.tile_pool(name="v", bufs=2))
            pt_p = ctx.enter_context(tc.tile_pool(name="pt", bufs=5))
            sm_p = ctx.enter_context(tc.tile_pool(name="sm", bufs=3))
            yst_p = ctx.enter_context(tc.tile_pool(name="yst", bufs=3))
            ps_mm = ctx.enter_context(
                tc.tile_pool(name="ps_mm", bufs=2, space="PSUM"))
            ps_st = ctx.enter_context(
                tc.tile_pool(name="ps_st", bufs=2, space="PSUM"))
            ps_av = ctx.enter_context(
                tc.tile_pool(name="ps_av", bufs=2, space="PSUM"))

            # constants (tiny; first on the queue)
            trimaskT = res.tile([128, 128], f32, tag="trimaskT")
            identb = res.tile([128, 128], bf16, tag="identb")
            ones_sq = res.tile([128, 128], bf16, tag="ones_sq")
            bq = res.tile([128, 3 * nh], f32, tag="bq")
            nc.sync.dma_start(trimaskT, tm_d)
            nc.sync.dma_start(identb, id_d)
            nc.sync.dma_start(ones_sq, oc_d)
            nc.sync.dma_start(bq, bq_d)

            wq = {}

            def load_wq(h):
                # weight stripes go out on the ACT hwdge queue so they are
                # not serialized behind the SP queue's x/output traffic
                for p in range(3):
                    hp = p * nh + h
                    w = w_p.tile([128, KT, 128], bf16, tag=f"w{p}",
                                 name=f"wq{hp}")
                    nc.scalar.dma_start(w, wq_d[hp * 128:(hp + 1) * 128, :])
                    wq[hp] = w

            load_wq(0)

            # x stripes
            xT = res.tile([128, KT, S], bf16, tag="xT")
            for kt in range(KT):
                nc.sync.dma_start(xT[:, kt, :], x_d[kt * 128:(kt + 1) * 128, :])

            wp3 = res.tile([128, nh, D], bf16, tag="wp3")
            for h in range(nh):
                nc.scalar.dma_start(wp3[:, h, :], wp_d[h * 128:(h + 1) * 128, :])

            OT = res.tile([128, nh, S], bf16, tag="OT")

            def alloc_qkv_dsts(h):
                return [qk_p.tile([128, S], bf16, tag=("qt", "kt_", "vt")[p],
                                  name=f"{('qt', 'kt', 'vt')[p]}{h}")
                        for p in range(3)]

            def qkv_copyout(h, p, pss, spc, nsp, dst):
                for i in range(nsp):
                    sp = spc + i
                    nc.vector.tensor_scalar(
                        out=dst[:, sp * span:(sp + 1) * span], in0=pss[i],
                        scalar1=bq[:, (p * nh + h):(p * nh + h) + 1],
                        scalar2=None, op0=Alu.add)

            def emit_qkv_group(h, p, spc, dsts):
                """One PSUM group pair: projection p, spans spc..spc+1."""
                w = wq[p * nh + h]
                nsp = min(2, NS - spc)
                pss = [ps_mm.tile([128, span], f32, tag="mm",
                                  name=f"mm{h}_{p}_{spc}_{i}")
                       for i in range(nsp)]
                for kt in range(KT):
                    for i in range(nsp):
                        sp = spc + i
                        nc.tensor.matmul(
                            pss[i], w[:, kt, :],
                            xT[:, kt, sp * span:(sp + 1) * span],
                            start=(kt == 0), stop=(kt == KT - 1))
                qkv_copyout(h, p, pss, spc, nsp, dsts[p])

            def emit_qkv_head0(dsts):
                """Head 0 qkv, kt-major across 6 concurrent PSUM groups so
                the PE can start as soon as the first x stripes land."""
                if NS < 2:
                    for spc in range(0, NS, 2):
                        for p in range(3):
                            emit_qkv_group(0, p, spc, dsts)
                    return
                pss = {}
                for p in range(3):
                    for i in range(2):
                        pool = [ps_mm, ps_st, ps_av][p]
                        tag = ["mm", "st", "o"][p] if p < 2 else \
                            ("o" if i == 0 else "s")
                        pss[(p, i)] = pool.tile([128, span], f32, tag=tag,
                                                name=f"h0mm{p}_{i}")
                for kt in range(KT):
                    for p in range(3):
                        w = wq[p * nh]
                        for i in range(2):
                            nc.tensor.matmul(
                                pss[(p, i)], w[:, kt, :],
                                xT[:, kt, i * span:(i + 1) * span],
                                start=(kt == 0), stop=(kt == KT - 1))
                for p in range(3):
                    qkv_copyout(0, p, [pss[(p, 0)], pss[(p, 1)]], 0, 2,
                                dsts[p])
                for spc in range(2, NS, 2):
                    for p in range(3):
                        emit_qkv_group(0, p, spc, dsts)

            def emit_vtrans(h, VT, vh):
                """vh[128, TT, 128] = natural-layout V via PE transpose."""
                for tg in range(0, TT, 4):
                    n = min(4, TT - tg)
                    psf = ps_mm.tile([128, span], f32, tag="mm",
                                     name=f"tp{h}_{tg}")
                    pst = psf.bitcast(bf16)
                    for j in range(n):
                        nc.tensor.transpose(
                            pst[:, j * 128:(j + 1) * 128],
                            VT[:, (tg + j) * 128:(tg + j + 1) * 128], identb)
                    nc.vector.tensor_copy(vh[:, tg:tg + n, :],
                                          pst[:, :n * 128])

            def emit_attention(h, sp, dsts, vh):
                QT, KTt, _ = dsts
                nkj = KPS * (sp + 1)
                ps_o = ps_av.tile([128, span], f32, tag="o", name=f"o{h}_{sp}")
                ps_s = ps_av.tile([128, span], f32, tag="s", name=f"s{h}_{sp}")
                pend = []

                def flush_one():
                    kj, pt, own = pend.pop(0)
                    st, en = (kj == 0), (kj == nkj - 1)
                    nc.tensor.matmul(
                        ps_s[:, own:], ones_sq, pt[:, own:],
                        start=st, stop=en)
                    nc.tensor.matmul(
                        ps_o[:, own:], vh[:, kj, :], pt[:, own:],
                        start=st, stop=en)

                for kj in range(nkj):
                    own = max(0, kj - KPS * sp) * 128
                    ps = ps_st.tile([128, span], f32, tag="st",
                                    name=f"st{h}_{sp}_{kj}")
                    nc.tensor.matmul(
                        ps[:, own:], KTt[:, kj * 128:(kj + 1) * 128],
                        QT[:, sp * span + own:(sp + 1) * span],
                        start=True, stop=True)
                    if kj >= KPS * sp:  # diagonal block: causal mask
                        nc.vector.tensor_tensor(
                            out=ps[:, own:own + 128],
                            in0=ps[:, own:own + 128],
                            in1=trimaskT, op=Alu.add)
                    pt = pt_p.tile([128, span], bf16, tag="pt",
                                   name=f"pt{h}_{sp}_{kj}")
                    nc.scalar.activation(
                        pt[:, own:], ps[:, own:], Act.Exp, scale=scale)
                    pend.append((kj, pt, own))
                    if len(pend) > 3:
                        flush_one()
                while pend:
                    flush_one()

                recipb = sm_p.tile([128, span], f32, tag="recipb",
                                   name=f"rb{h}_{sp}")
                nc.vector.reciprocal_approx_fast(out=recipb, in_=ps_s)
                nc.vector.tensor_tensor(
                    out=OT[:, h, sp * span:(sp + 1) * span],
                    in0=ps_o, in1=recipb, op=Alu.mult)

            def emit_proj_chunk(spc, dclo, dchi):
                nsp = min(2, NS - spc)
                for dc in range(dclo, dchi):
                    dsl = slice(dc * 128, (dc + 1) * 128)
                    # alternate PSUM pools for 4 banks in flight (st pool is
                    # idle during the projection tail)
                    pool, tag = (ps_mm, "mm") if dc % 2 == 0 else (ps_st, "st")
                    pss = [pool.tile([128, span], f32, tag=tag,
                                     name=f"pj{spc}_{dc}_{i}")
                           for i in range(nsp)]
                    for hh in range(nh):
                        for i in range(nsp):
                            sp = spc + i
                            nc.tensor.matmul(
                                pss[i], wp3[:, hh, dsl],
                                OT[:, hh, sp * span:(sp + 1) * span],
                                start=(hh == 0), stop=(hh == nh - 1))
                    yst = yst_p.tile([128, nsp * span], bf16, tag="yst",
                                     name=f"yst{spc}_{dc}")
                    for i in range(nsp):
                        seg = yst[:, i * span:(i + 1) * span]
                        if dc % 2 == 0:
                            nc.scalar.copy(seg, pss[i])
                        else:
                            nc.vector.tensor_copy(seg, pss[i])
                    nc.sync.dma_start(
                        yt_d[dc * 128:(dc + 1) * 128,
                             spc * span:(spc + nsp) * span], yst)

            # ---- software-pipelined schedule ----
            NDC = D // 128
            dsts = alloc_qkv_dsts(0)
            emit_qkv_head0(dsts)
            vh = v_p.tile([128, TT, 128], bf16, tag="v", name="vh0")
            emit_vtrans(0, dsts[2], vh)

            for h in range(nh):
                fillers = [[] for _ in range(NS)]
                if h + 1 < nh:
                    load_wq(h + 1)
                    nxt = alloc_qkv_dsts(h + 1)
                    nxtvh = v_p.tile([128, TT, 128], bf16, tag="v",
                                     name=f"vh{h + 1}")
                    work = [(emit_qkv_group, (h + 1, p, spc, nxt))
                            for spc in range(0, NS, 2) for p in range(3)]
                    work.append((emit_vtrans, (h + 1, nxt[2], nxtvh)))
                    per = (len(work) + NS - 1) // NS
                    for sp in range(NS):
                        lo = sp * per
                        fillers[sp] = work[lo:lo + per] if sp < NS - 1 \
                            else work[lo:]
                else:
                    # output projection: chunk (spc, dc-range) becomes legal
                    # once span spc+nsp-1 of the last head is done
                    for spc in range(0, NS, 2):
                        nsp = min(2, NS - spc)
                        rdy = spc + nsp - 1
                        if rdy == NS - 1:   # final spans: emit whole chunk
                            fillers[rdy].append(
                                (emit_proj_chunk, (spc, 0, NDC)))
                        else:               # split across two spans
                            fillers[rdy].append(
                                (emit_proj_chunk, (spc, 0, NDC // 2)))
                            fillers[min(rdy + 1, NS - 1)].append(
                                (emit_proj_chunk, (spc, NDC // 2, NDC)))
                for sp in range(NS):
                    emit_attention(h, sp, dsts, vh)
                    for fn, args in fillers[sp]:
                        fn(*args)
                if h + 1 < nh:
                    dsts, vh = nxt, nxtvh

    nc.finalize()
    return nc


def _prep_core_inputs(x, W_qkv, b_qkv, W_proj, core, S=S, D=D, nh=HPC,
                      b=None, hg=None):
    import ml_dtypes
    bf16 = ml_dtypes.bfloat16
    ngr = NCORES // B
    if b is None:
        b, hg = core // ngr, core % ngr
    KT = D // 128
    Dfull = W_qkv.shape[1] // 3

    wq = np.empty((3 * nh * 128, D), dtype=bf16)
    bqt = np.zeros((128, 3 * nh), dtype=np.float32)
    for p in range(3):
        for h in range(nh):
            g = hg * nh + h
            col = p * Dfull + g * 128
            blk = W_qkv[:, col:col + 128]            # [D, 128]
            hp = p * nh + h
            wq[hp * 128:(hp + 1) * 128] = (
                blk.reshape(KT, 128, 128).transpose(1, 0, 2).reshape(128, D)
                .astype(bf16))
            bqt[:, hp] = b_qkv[col:col + 128]
    wp = W_proj[hg * nh * 128:(hg + 1) * nh * 128, :].astype(bf16)

    r = np.arange(128)
    trimaskT = np.where(r[:, None] <= r[None, :], 0.0, NEG).astype(np.float32)
    return {
        "xt": np.ascontiguousarray(x[b].T).astype(bf16),
        "wqkv": wq,
        "bqkv": bqt,
        "wproj": wp,
        "trimaskT": trimaskT,
        "identb": np.eye(128, dtype=bf16),
        "ones_sq": np.ones((128, 128), dtype=bf16),
    }


_CACHE = {}


def kernel(x, W_qkv, b_qkv, W_proj, b_proj, mask):
    from concourse.bass_utils import run_bass_kernel_spmd

    x = np.asarray(x)
    W_qkv = np.asarray(W_qkv)
    b_qkv = np.asarray(b_qkv)
    W_proj = np.asarray(W_proj)
    b_proj = np.asarray(b_proj)

    if "nc" not in _CACHE:
        _CACHE["nc"] = build_nc()
    nc = _CACHE["nc"]

    in_maps = [_prep_core_inputs(x, W_qkv, b_qkv, W_proj, c)
               for c in range(NCORES)]
    res = run_bass_kernel_spmd(nc, in_maps, core_ids=list(range(NCORES)))

    ngr = NCORES // B
    out = np.empty((B, S, D), dtype=np.float32)
    for b in range(B):
        acc = res.results[b * ngr]["yt"].astype(np.float32)
        for g in range(1, ngr):
            acc = acc + res.results[b * ngr + g]["yt"].astype(np.float32)
        out[b] = acc.T + b_proj[None, :]
    return out
